# revision 3
# baseline (speedup 1.0000x reference)
"""Trainium2 Bass kernel for EntityAttention.

    beta[b,e,a] = (agent[b,e] @ w_psi) . (vis[b,e,a] @ w_phi)
    out         = softmax_a(beta)

Refactored so the huge `visible_observations` tensor is read exactly once,
in its natural layout, by a fused multiply+cumulative-sum on the Vector
engine (a custom DVE op: out = cumsum(in0 * in1)); per-a segment sums are
recovered by differencing the cumsum at segment boundaries:

    qT[k, be]   = sum_din w_psi[din, k] * agent[be, din]      (PE)
    t[be, dout] = sum_k   qT[k, be]     * w_phiT[k, dout]     (PE)
    cum         = cumsum_f(vis[be, (a,d)] * t[be, d bcast])   (DVE, 1 op / 8 a)
    beta[be, a] = cum[.., a*D+D-1] - cum[.., a*D-1]           (GpSimd, tiny)
    out[be, a]  = softmax_a(beta)                             (DVE max + ACT
                                                               exp + GpSimd
                                                               normalize)

Sharding: data-parallel over the batch axis across 8 NeuronCores
(16 batches / core); w_psi / w_phi replicated.

Engine budget in steady state: the DVE queue is (almost) pure scans so it
keeps pace with the ~430 GB/s HBM stream; beta extraction and softmax
normalization run on the otherwise-idle GpSimd engine; PSUM evacuations on
the chunk-0 critical path run on the (idle until first scan) Vector engine,
later chunks' on Scalar.
"""

from contextlib import ExitStack

import numpy as np

import concourse.bass as bass
import concourse.tile as tile
from concourse import bacc, bass_utils, dve_ops, mybir
from concourse.dve_spec import AluOp, Spec, Src0, Src1, _has_src1, lower, scan
from concourse.dve_uop import DveOpSpec
from concourse.masks import make_identity

# Problem shape (hardcoded per contract; kernel.py must be self-contained).
B, E, A, D, K = 128, 32, 16, 512, 128
N_CORES = 8
B_SH = B // N_CORES          # batches per core = 16
BE = B_SH * E                # rows per core = 512
NBC = BE // 128              # be-chunks of 128 partitions = 4
NDC = D // 128               # din-chunks = 4
HALF_A = 8                   # visible-agents per streamed tile (max)
F32 = mybir.dt.float32


# ---- custom DVE op: out = cumsum(in0 * in1) along the free axis ---------- #

def _ref_cumsum_mul(in0, in1, s0, s1, imm2):
    p = in0.shape[0]
    a = np.asarray(in0, np.float32).reshape(p, -1)
    b = np.ascontiguousarray(np.asarray(in1, np.float32)).reshape(p, -1)
    if b.shape[1] != a.shape[1]:
        b = np.tile(b, (1, a.shape[1] // b.shape[1]))
    init = s0 if isinstance(s0, np.ndarray) else np.float32(s0)
    return init + np.cumsum(a * b, axis=-1, dtype=np.float32)


def _register_cumsum_mul():
    name = "CUMSUM_MUL_ANT"
    if name in dve_ops._SUB_OPCODE_FOR_NAME:
        return next(op for op in dve_ops.OPS if op.name == name)
    from concourse.dve_spec import C0
    spec = Spec(body=scan(AluOp.ADD, Src0 * Src1, init=C0), reference=_ref_cumsum_mul)
    row = dve_ops._CUSTOM_DVE_ROW_BASE + len(dve_ops.OPS)
    assert row < 0x20
    shas = {}
    for ver in ("v3", "v4"):
        d = DveOpSpec(name=name, opcode=row, uops=lower(spec, ver=ver),
                      rd1_en=_has_src1(spec))
        shas[ver] = d.sha(ver)
    op = dve_ops.DveOp(name, spec, subdim=False, uops_sha=shas)
    dve_ops._SUB_OPCODE_FOR_NAME[name] = row
    dve_ops.OPS.append(op)
    dve_ops.CUSTOM_DVE_SPECS[name] = spec
    return op


CUMSUM_MUL = _register_cumsum_mul()


def _bcast_mid(ap_2d, count):
    """[P, N] AP -> [P, count, N] AP with a step-0 middle dim."""
    return bass.AP(
        tensor=ap_2d.tensor,
        offset=ap_2d.offset,
        ap=[ap_2d.ap[0], [0, count], *ap_2d.ap[1:]],
    )


def _emit(tc, nc, ag_d, vis_d, wpsi_d, wphi_d, out_d):
    with ExitStack() as ctx:
        const = ctx.enter_context(tc.tile_pool(name="const", bufs=1))
        agp = ctx.enter_context(tc.tile_pool(name="agp", bufs=1))
        visp = ctx.enter_context(tc.tile_pool(name="visp", bufs=8))
        small = ctx.enter_context(tc.tile_pool(name="small", bufs=4))
        ps_tr = ctx.enter_context(tc.tile_pool(name="ps_tr", bufs=4, space="PSUM"))
        ps_mm = ctx.enter_context(tc.tile_pool(name="ps_mm", bufs=2, space="PSUM"))

        ident = const.tile([128, 128], F32)
        make_identity(nc, ident)

        # DMA issue order on the SP (sync) HWDGE ring IS the stream order.
        # The three t[0]-gating loads go first; the 16 MB vis stream follows
        # immediately and runs at line rate behind them.
        # Weights use interleaved din chunking (chunk r = rows d % 4 == r)
        # so each partition line is a contiguous 2 KB DMA.
        ag_tiles = {}
        ag_tiles[0] = agp.tile([128, D], F32, tag="ag0", name="ag0")
        nc.sync.dma_start(out=ag_tiles[0], in_=ag_d[0:128, :])
        wpsi_sb = const.tile([128, NDC, K], F32)
        nc.sync.dma_start(out=wpsi_sb, in_=wpsi_d.rearrange("(p r) k -> p r k", r=NDC))
        wphi_sb = const.tile([128, NDC, K], F32)
        nc.sync.dma_start(out=wphi_sb, in_=wphi_d.rearrange("(p r) k -> p r k", r=NDC))

        # vis DMA issue pass. SP issues each dma_start in ~0.7 us; agent
        # chunk c is issued just ahead of the vis tiles whose scans need
        # t[c]. Last chunk uses finer tiles to shorten the pipeline tail.
        chunk_groups = {0: [8, 8], 1: [8, 8], 2: [8, 8], 3: [8, 4, 2, 1, 1]}
        vis_tiles = {}
        for c in range(NBC):
            cs = slice(c * 128, (c + 1) * 128)
            if c > 0:
                ag_tiles[c] = agp.tile([128, D], F32, tag=f"ag{c}", name=f"ag{c}")
                nc.sync.dma_start(out=ag_tiles[c], in_=ag_d[cs, :])
            a0 = 0
            for gi, na in enumerate(chunk_groups[c]):
                vis_sb = visp.tile([128, HALF_A, D], F32, tag="vis",
                                   name=f"vis{c}_{gi}")[:, :na, :]
                nc.sync.dma_start(
                    out=vis_sb, in_=vis_d[cs, a0 * D:(a0 + na) * D]
                )
                vis_tiles[(c, gi)] = (vis_sb, a0, na)
                a0 += na

        # Warm the PE clock (HAM) with dummy transposes, bridging the gap
        # until ag0 lands so the governor sees sustained PE load and the
        # t[0] chain runs above the cold 1.2 GHz.
        for wup in range(9):
            warm_ps = ps_tr.tile([128, 128], F32, tag="tr", name=f"warm{wup}")
            nc.tensor.transpose(warm_ps, ident, ident)

        # PE prologue per chunk: agT transposes -> qT -> t.  Chunk 0 is the
        # latency-critical chain gating the first scan: its PSUM
        # evacuations run on the (idle until then) Vector engine; later
        # chunks' run on Scalar so the DVE queue stays scans-only.
        # w_phiT with natural dout order: wphiT4[k, dl, r] = w_phi[4*dl+r, k],
        # flat free index f = dl*4 + r = dout.
        agT_sb = const.tile([128, NDC, BE], F32)
        qT_sb = const.tile([128, BE], F32)
        wphiT_sb = const.tile([128, 128, NDC], F32)
        t_tiles = []
        for c in range(NBC):
            cs = slice(c * 128, (c + 1) * 128)
            copy_eng = nc.vector.tensor_copy if c == 0 else nc.scalar.copy
            ag_v = ag_tiles[c].rearrange("p (q r) -> p q r", r=NDC)
            for r in range(NDC):
                tr_ps = ps_tr.tile([128, 128], F32, tag="tr", name=f"tra{c}_{r}")
                nc.tensor.transpose(tr_ps, ag_v[:, :, r], ident)
                copy_eng(agT_sb[:, r, cs], tr_ps)
            # qT[:, cs] = sum_r w_psi_chunk_r.T @ agT_chunk_r
            qt_ps = ps_mm.tile([128, 128], F32, tag="qt", name=f"qt{c}")
            for r in range(NDC):
                nc.tensor.matmul(
                    qt_ps,
                    lhsT=wpsi_sb[:, r, :],
                    rhs=agT_sb[:, r, cs],
                    start=(r == 0),
                    stop=(r == NDC - 1),
                )
            copy_eng(qT_sb[:, cs], qt_ps)
            if c == 0:
                # wphiT transposes sit between qT and the t matmul on the PE
                # queue: they are only needed for t, and this keeps the agT
                # chain (gated by the ag0 DMA) at the front of the queue.
                for r in range(NDC):
                    tr_ps = ps_tr.tile([128, 128], F32, tag="tr", name=f"trw{r}")
                    nc.tensor.transpose(tr_ps, wphi_sb[:, r, :], ident)
                    nc.scalar.copy(wphiT_sb[:, :, r], tr_ps)
            # t[be_c, dout] = qT[:, cs].T @ w_phiT
            t_ps = ps_mm.tile([128, D], F32, tag="t", name=f"tps{c}")
            nc.tensor.matmul(
                t_ps, lhsT=qT_sb[:, cs], rhs=wphiT_sb[:, :, :], start=True, stop=True
            )
            t_sb = const.tile([128, D], F32, tag=f"t{c}", name=f"t{c}")
            copy_eng(t_sb, t_ps)
            t_tiles.append(t_sb)

        # Fused multiply+cumsum (in place, per DMA tile) on the DVE; the
        # boundary differencing runs on GpSimd so scans stay back-to-back;
        # softmax = DVE max + ACT exp(accum) + GpSimd normalize_recip.
        for c in range(NBC):
            cs = slice(c * 128, (c + 1) * 128)
            beta_sb = small.tile([128, A], F32, tag="beta", name=f"beta{c}")
            for gi in range(len(chunk_groups[c])):
                vis_sb, a0, na = vis_tiles[(c, gi)]
                nc.vector._custom_dve(
                    CUMSUM_MUL,
                    out=vis_sb,
                    in0=vis_sb,
                    in1=_bcast_mid(t_tiles[c], na),
                )
                # boundary values cum[:, s, D-1] -> [128, na]
                bnd = vis_sb[:, :, D - 1:D].rearrange("p s o -> p (s o)")
                nc.gpsimd.tensor_copy(beta_sb[:, a0:a0 + 1], bnd[:, 0:1])
                if na > 1:
                    nc.gpsimd.tensor_sub(
                        beta_sb[:, a0 + 1:a0 + na], bnd[:, 1:na], bnd[:, 0:na - 1],
                    )
            negm = small.tile([128, 1], F32, tag="negm", name=f"negm{c}")
            nc.vector.tensor_reduce(
                negm, beta_sb, axis=mybir.AxisListType.X,
                op=mybir.AluOpType.max, negate=True,
            )
            prob = small.tile([128, A], F32, tag="prob", name=f"prob{c}")
            ssum = small.tile([128, 1], F32, tag="ssum", name=f"ssum{c}")
            nc.scalar.activation(
                prob, beta_sb, mybir.ActivationFunctionType.Exp,
                bias=negm, scale=1.0, accum_out=ssum,
            )
            osb = small.tile([128, A], F32, tag="osb", name=f"osb{c}")
            nc.gpsimd.normalize_recip(osb, prob, ssum)
            nc.scalar.dma_start(out=out_d[cs, :], in_=osb)


def _build_program():
    nc = bacc.Bacc("TRN2", target_bir_lowering=False, debug=False)
    ag_d = nc.dram_tensor("agent", (BE, D), F32, kind="ExternalInput").ap()
    vis_d = nc.dram_tensor("vis", (BE, A * D), F32, kind="ExternalInput").ap()
    wpsi_d = nc.dram_tensor("w_psi", (D, K), F32, kind="ExternalInput").ap()
    wphi_d = nc.dram_tensor("w_phi", (D, K), F32, kind="ExternalInput").ap()
    out_d = nc.dram_tensor("out", (BE, A), F32, kind="ExternalOutput").ap()
    with tile.TileContext(nc) as tc:
        _emit(tc, nc, ag_d, vis_d, wpsi_d, wphi_d, out_d)
    nc.compile()
    return nc


_PROG = None


def _get_program():
    global _PROG
    if _PROG is None:
        _PROG = _build_program()
    return _PROG


def make_in_maps(agent_observation, visible_observations, w_psi, w_phi):
    agent = np.ascontiguousarray(np.asarray(agent_observation, np.float32)).reshape(B, E, D)
    vis = np.ascontiguousarray(np.asarray(visible_observations, np.float32)).reshape(B, E, A, D)
    wpsi = np.ascontiguousarray(np.asarray(w_psi, np.float32))
    wphi = np.ascontiguousarray(np.asarray(w_phi, np.float32))
    in_maps = []
    for ci in range(N_CORES):
        sl = slice(ci * B_SH, (ci + 1) * B_SH)
        in_maps.append({
            "agent": np.ascontiguousarray(agent[sl].reshape(BE, D)),
            "vis": np.ascontiguousarray(vis[sl].reshape(BE, A * D)),
            "w_psi": wpsi,
            "w_phi": wphi,
        })
    return in_maps


def run_sharded(in_maps, trace=False, **kwargs):
    nc = _get_program()
    return bass_utils.run_bass_kernel_spmd(
        nc, in_maps, core_ids=list(range(N_CORES)), trace=trace, **kwargs
    )


def kernel(agent_observation, visible_observations, w_psi, w_phi):
    in_maps = make_in_maps(agent_observation, visible_observations, w_psi, w_phi)
    res = run_sharded(in_maps)
    return np.concatenate(
        [r["out"].reshape(B_SH, E, A) for r in res.results], axis=0
    )


# revision 4
# speedup vs baseline: 1.3564x; 1.3564x over previous
"""Trainium2 Bass kernel for EntityAttention.

    beta[b,e,a] = (agent[b,e] @ w_psi) . (vis[b,e,a] @ w_phi)
    out         = softmax_a(beta)

Refactored so the huge `visible_observations` tensor is read exactly once,
in its natural layout, by a fused multiply+cumulative-sum on the Vector
engine (a custom DVE op: out = cumsum(in0 * in1)); per-a segment sums are
recovered by differencing the cumsum at segment boundaries:

    qT[k, be]   = sum_din w_psi[din, k] * agent[be, din]      (PE)
    t[be, dout] = sum_k   qT[k, be]     * w_phiT[k, dout]     (PE)
    cum         = cumsum_f(vis[be, (a,d)] * t[be, d bcast])   (DVE, 1 op / 8 a)
    beta[be, a] = cum[.., a*D+D-1] - cum[.., a*D-1]           (GpSimd, tiny)
    out[be, a]  = softmax_a(beta)                             (DVE max + ACT
                                                               exp + GpSimd
                                                               normalize)

Sharding: data-parallel over the batch axis across 8 NeuronCores
(16 batches / core); w_psi / w_phi replicated.

Engine budget in steady state: the DVE queue is (almost) pure scans so it
keeps pace with the ~430 GB/s HBM stream; beta extraction and softmax
normalization run on the otherwise-idle GpSimd engine; PSUM evacuations on
the chunk-0 critical path run on the (idle until first scan) Vector engine,
later chunks' on Scalar.
"""

from contextlib import ExitStack

import numpy as np

import concourse.bass as bass
import concourse.tile as tile
from concourse import bacc, bass_utils, dve_ops, mybir
from concourse.dve_spec import AluOp, Spec, Src0, Src1, _has_src1, lower, scan
from concourse.dve_uop import DveOpSpec
from concourse.masks import make_identity

# Problem shape (hardcoded per contract; kernel.py must be self-contained).
B, E, A, D, K = 128, 32, 16, 512, 128
N_CORES = 8
B_SH = B // N_CORES          # batches per core = 16
BE = B_SH * E                # rows per core = 512
NBC = BE // 128              # be-chunks of 128 partitions = 4
NDC = D // 128               # din-chunks = 4
HALF_A = 8                   # visible-agents per streamed tile (max)
F32 = mybir.dt.float32


# ---- custom DVE op: out = cumsum(in0 * in1) along the free axis ---------- #

def _ref_cumsum_mul(in0, in1, s0, s1, imm2):
    p = in0.shape[0]
    a = np.asarray(in0, np.float32).reshape(p, -1)
    b = np.ascontiguousarray(np.asarray(in1, np.float32)).reshape(p, -1)
    if b.shape[1] != a.shape[1]:
        b = np.tile(b, (1, a.shape[1] // b.shape[1]))
    init = s0 if isinstance(s0, np.ndarray) else np.float32(s0)
    return init + np.cumsum(a * b, axis=-1, dtype=np.float32)


def _register_cumsum_mul():
    name = "CUMSUM_MUL_ANT"
    if name in dve_ops._SUB_OPCODE_FOR_NAME:
        return next(op for op in dve_ops.OPS if op.name == name)
    from concourse.dve_spec import C0
    spec = Spec(body=scan(AluOp.ADD, Src0 * Src1, init=C0), reference=_ref_cumsum_mul)
    row = dve_ops._CUSTOM_DVE_ROW_BASE + len(dve_ops.OPS)
    assert row < 0x20
    shas = {}
    for ver in ("v3", "v4"):
        d = DveOpSpec(name=name, opcode=row, uops=lower(spec, ver=ver),
                      rd1_en=_has_src1(spec))
        shas[ver] = d.sha(ver)
    op = dve_ops.DveOp(name, spec, subdim=False, uops_sha=shas)
    dve_ops._SUB_OPCODE_FOR_NAME[name] = row
    dve_ops.OPS.append(op)
    dve_ops.CUSTOM_DVE_SPECS[name] = spec
    return op


CUMSUM_MUL = _register_cumsum_mul()


def _bcast_mid(ap_2d, count):
    """[P, N] AP -> [P, count, N] AP with a step-0 middle dim."""
    return bass.AP(
        tensor=ap_2d.tensor,
        offset=ap_2d.offset,
        ap=[ap_2d.ap[0], [0, count], *ap_2d.ap[1:]],
    )


def _emit(tc, nc, ag_d, vis_d, wpsi_d, wphi_d, out_d):
    with ExitStack() as ctx:
        const = ctx.enter_context(tc.tile_pool(name="const", bufs=1))
        agp = ctx.enter_context(tc.tile_pool(name="agp", bufs=1))
        visp = ctx.enter_context(tc.tile_pool(name="visp", bufs=8))
        small = ctx.enter_context(tc.tile_pool(name="small", bufs=4))
        ps_tr = ctx.enter_context(tc.tile_pool(name="ps_tr", bufs=4, space="PSUM"))
        ps_mm = ctx.enter_context(tc.tile_pool(name="ps_mm", bufs=2, space="PSUM"))

        ident = const.tile([128, 128], F32)
        make_identity(nc, ident)
        # Pay GpSimd's one-time TENSOR_TENSOR ucode load (~4 us) now, while
        # the Pool engine is idle, not at the first beta extraction.
        scratch = small.tile([128, 2], F32, tag="scr", name="scr")
        nc.gpsimd.tensor_sub(scratch[:, 0:1], ident[:, 0:1], ident[:, 1:2])
        nc.gpsimd.tensor_copy(scratch[:, 1:2], ident[:, 0:1])

        # DMA issue order on the SP (sync) HWDGE ring IS the stream order:
        # the ring drains FIFO, so byte position in the issue sequence is
        # arrival time.  The t[0]-gating loads go first (ag0/wpsi/wphi),
        # then ag1 (t[1] is needed ~10 us in), then the vis stream with
        # ag2/ag3 slotted after chunk 0 (their t's are needed much later).
        # Weights use interleaved din chunking (chunk r = rows d % 4 == r)
        # so each partition line is a contiguous 2 KB DMA.
        ag_tiles = {}
        for c in range(NBC):
            ag_tiles[c] = agp.tile([128, D], F32, tag=f"ag{c}", name=f"ag{c}")
        nc.sync.dma_start(out=ag_tiles[0], in_=ag_d[0:128, :])
        wpsi_sb = const.tile([128, NDC, K], F32)
        nc.sync.dma_start(out=wpsi_sb, in_=wpsi_d.rearrange("(p r) k -> p r k", r=NDC))
        wphi_sb = const.tile([128, NDC, K], F32)
        nc.sync.dma_start(out=wphi_sb, in_=wphi_d.rearrange("(p r) k -> p r k", r=NDC))
        nc.sync.dma_start(out=ag_tiles[1], in_=ag_d[128:256, :])

        # vis DMA issue pass. First/last chunks use finer tiles to shorten
        # the pipeline ramp and tail.
        chunk_groups = {0: [4, 4, 8], 1: [8, 8], 2: [8, 8], 3: [8, 4, 2, 2]}
        vis_tiles = {}
        for c in range(NBC):
            cs = slice(c * 128, (c + 1) * 128)
            a0 = 0
            for gi, na in enumerate(chunk_groups[c]):
                vis_sb = visp.tile([128, HALF_A, D], F32, tag="vis",
                                   name=f"vis{c}_{gi}")[:, :na, :]
                nc.sync.dma_start(
                    out=vis_sb, in_=vis_d[cs, a0 * D:(a0 + na) * D]
                )
                vis_tiles[(c, gi)] = (vis_sb, a0, na)
                a0 += na
            if c == 0:
                nc.sync.dma_start(out=ag_tiles[2], in_=ag_d[256:384, :])
                nc.sync.dma_start(out=ag_tiles[3], in_=ag_d[384:512, :])

        # Warm the PE clock (HAM) with dummy transposes, bridging the gap
        # until ag0 lands so the governor sees sustained PE load and the
        # t[0] chain runs above the cold 1.2 GHz.
        for wup in range(7):
            warm_ps = ps_tr.tile([128, 128], F32, tag="tr", name=f"warm{wup}")
            nc.tensor.transpose(warm_ps, ident, ident)

        # PE prologue per chunk: agT transposes -> qT -> t.  Chunk 0 is the
        # latency-critical chain gating the first scan: its PSUM
        # evacuations run on the (idle until then) Vector engine; later
        # chunks' run on Scalar so the DVE stays free for scans.
        # w_phiT with natural dout order: wphiT4[k, dl, r] = w_phi[4*dl+r, k],
        # flat free index f = dl*4 + r = dout.
        agT_sb = const.tile([128, NDC, BE], F32)
        qT_sb = const.tile([128, BE], F32)
        wphiT_sb = const.tile([128, 128, NDC], F32)
        t_tiles = []
        for c in range(NBC):
            cs = slice(c * 128, (c + 1) * 128)
            copy_eng = nc.vector.tensor_copy if c == 0 else nc.scalar.copy
            ag_v = ag_tiles[c].rearrange("p (q r) -> p q r", r=NDC)
            for r in range(NDC):
                tr_ps = ps_tr.tile([128, 128], F32, tag="tr", name=f"tra{c}_{r}")
                nc.tensor.transpose(tr_ps, ag_v[:, :, r], ident)
                copy_eng(agT_sb[:, r, cs], tr_ps)
            # qT[:, cs] = sum_r w_psi_chunk_r.T @ agT_chunk_r
            qt_ps = ps_mm.tile([128, 128], F32, tag="qt", name=f"qt{c}")
            for r in range(NDC):
                nc.tensor.matmul(
                    qt_ps,
                    lhsT=wpsi_sb[:, r, :],
                    rhs=agT_sb[:, r, cs],
                    start=(r == 0),
                    stop=(r == NDC - 1),
                )
            copy_eng(qT_sb[:, cs], qt_ps)
            if c == 0:
                # wphiT transposes sit between qT and the t matmul on the PE
                # queue: they are only needed for t, and this keeps the agT
                # chain (gated by the ag0 DMA) at the front of the queue.
                for r in range(NDC):
                    tr_ps = ps_tr.tile([128, 128], F32, tag="tr", name=f"trw{r}")
                    nc.tensor.transpose(tr_ps, wphi_sb[:, r, :], ident)
                    nc.scalar.copy(wphiT_sb[:, :, r], tr_ps)
            # t[be_c, dout] = qT[:, cs].T @ w_phiT
            t_ps = ps_mm.tile([128, D], F32, tag="t", name=f"tps{c}")
            nc.tensor.matmul(
                t_ps, lhsT=qT_sb[:, cs], rhs=wphiT_sb[:, :, :], start=True, stop=True
            )
            t_sb = const.tile([128, D], F32, tag=f"t{c}", name=f"t{c}")
            copy_eng(t_sb, t_ps)
            t_tiles.append(t_sb)

        # Streaming phase.  DVE: scans (plus one tiny reduce/reciprocal per
        # chunk, emitted one chunk late so they never block the queue head).
        # GpSimd: boundary differencing — it is the recycled vis tile's last
        # reader, and nothing else ever sits in the Pool queue, so the WAR
        # release for the DMA ring is prompt.  Scalar: exp + normalize + out.
        beta_t, negm_t, prob_t, ssum_t, osb_t = {}, {}, {}, {}, {}
        for c in range(NBC):
            beta_t[c] = small.tile([128, A], F32, tag="beta", name=f"beta{c}")
            negm_t[c] = small.tile([128, 1], F32, tag="negm", name=f"negm{c}")
            prob_t[c] = small.tile([128, A], F32, tag="prob", name=f"prob{c}")
            ssum_t[c] = small.tile([128, 1], F32, tag="ssum", name=f"ssum{c}")
            osb_t[c] = small.tile([128, A], F32, tag="osb", name=f"osb{c}")

        def softmax_head(c):
            nc.vector.tensor_reduce(
                negm_t[c], beta_t[c], axis=mybir.AxisListType.X,
                op=mybir.AluOpType.max, negate=True,
            )
            nc.scalar.activation(
                prob_t[c], beta_t[c], mybir.ActivationFunctionType.Exp,
                bias=negm_t[c], scale=1.0, accum_out=ssum_t[c],
            )

        def softmax_tail(c):
            rec = small.tile([128, 1], F32, tag="rec", name=f"rec{c}")
            nc.vector.reciprocal(rec, ssum_t[c])
            nc.scalar.mul(osb_t[c], prob_t[c], rec)
            cs = slice(c * 128, (c + 1) * 128)
            nc.scalar.dma_start(out=out_d[cs, :], in_=osb_t[c])

        for c in range(NBC):
            for gi in range(len(chunk_groups[c])):
                vis_sb, a0, na = vis_tiles[(c, gi)]
                nc.vector._custom_dve(
                    CUMSUM_MUL,
                    out=vis_sb,
                    in0=vis_sb,
                    in1=_bcast_mid(t_tiles[c], na),
                )
                # boundary values cum[:, s, D-1] -> [128, na]
                bnd = vis_sb[:, :, D - 1:D].rearrange("p s o -> p (s o)")
                nc.gpsimd.tensor_copy(beta_t[c][:, a0:a0 + 1], bnd[:, 0:1])
                if na > 1:
                    nc.gpsimd.tensor_sub(
                        beta_t[c][:, a0 + 1:a0 + na], bnd[:, 1:na], bnd[:, 0:na - 1],
                    )
                if c >= 1 and gi == 0:
                    softmax_head(c - 1)
                if c >= 1 and gi == 1:
                    softmax_tail(c - 1)
        softmax_head(NBC - 1)
        softmax_tail(NBC - 1)


def _build_program():
    nc = bacc.Bacc("TRN2", target_bir_lowering=False, debug=False)
    ag_d = nc.dram_tensor("agent", (BE, D), F32, kind="ExternalInput").ap()
    vis_d = nc.dram_tensor("vis", (BE, A * D), F32, kind="ExternalInput").ap()
    wpsi_d = nc.dram_tensor("w_psi", (D, K), F32, kind="ExternalInput").ap()
    wphi_d = nc.dram_tensor("w_phi", (D, K), F32, kind="ExternalInput").ap()
    out_d = nc.dram_tensor("out", (BE, A), F32, kind="ExternalOutput").ap()
    with tile.TileContext(nc) as tc:
        _emit(tc, nc, ag_d, vis_d, wpsi_d, wphi_d, out_d)
    nc.compile()
    return nc


_PROG = None


def _get_program():
    global _PROG
    if _PROG is None:
        _PROG = _build_program()
    return _PROG


def make_in_maps(agent_observation, visible_observations, w_psi, w_phi):
    agent = np.ascontiguousarray(np.asarray(agent_observation, np.float32)).reshape(B, E, D)
    vis = np.ascontiguousarray(np.asarray(visible_observations, np.float32)).reshape(B, E, A, D)
    wpsi = np.ascontiguousarray(np.asarray(w_psi, np.float32))
    wphi = np.ascontiguousarray(np.asarray(w_phi, np.float32))
    in_maps = []
    for ci in range(N_CORES):
        sl = slice(ci * B_SH, (ci + 1) * B_SH)
        in_maps.append({
            "agent": np.ascontiguousarray(agent[sl].reshape(BE, D)),
            "vis": np.ascontiguousarray(vis[sl].reshape(BE, A * D)),
            "w_psi": wpsi,
            "w_phi": wphi,
        })
    return in_maps


def run_sharded(in_maps, trace=False, **kwargs):
    nc = _get_program()
    return bass_utils.run_bass_kernel_spmd(
        nc, in_maps, core_ids=list(range(N_CORES)), trace=trace, **kwargs
    )


def kernel(agent_observation, visible_observations, w_psi, w_phi):
    in_maps = make_in_maps(agent_observation, visible_observations, w_psi, w_phi)
    res = run_sharded(in_maps)
    return np.concatenate(
        [r["out"].reshape(B_SH, E, A) for r in res.results], axis=0
    )


# revision 6
# speedup vs baseline: 1.3569x; 1.0004x over previous
"""Trainium2 Bass kernel for EntityAttention.

    beta[b,e,a] = (agent[b,e] @ w_psi) . (vis[b,e,a] @ w_phi)
    out         = softmax_a(beta)

Refactored so the huge `visible_observations` tensor is read exactly once,
in its natural layout, by a fused multiply+cumulative-sum on the Vector
engine (a custom DVE op: out = cumsum(in0 * in1)); per-a segment sums are
recovered by differencing the cumsum at segment boundaries:

    qT[k, be]   = sum_din w_psi[din, k] * agent[be, din]      (PE)
    t[be, dout] = sum_k   qT[k, be]     * w_phiT[k, dout]     (PE)
    cum         = cumsum_f(vis[be, (a,d)] * t[be, d bcast])   (DVE, 1 op / 8 a)
    beta[be, a] = cum[.., a*D+D-1] - cum[.., a*D-1]           (GpSimd, tiny)
    out[be, a]  = softmax_a(beta)                             (DVE max + ACT
                                                               exp + GpSimd
                                                               normalize)

Sharding: data-parallel over the batch axis across 8 NeuronCores
(16 batches / core); w_psi / w_phi replicated.

Engine budget in steady state: the DVE queue is (almost) pure scans so it
keeps pace with the ~430 GB/s HBM stream; beta extraction and softmax
normalization run on the otherwise-idle GpSimd engine; PSUM evacuations on
the chunk-0 critical path run on the (idle until first scan) Vector engine,
later chunks' on Scalar.
"""

from contextlib import ExitStack

import numpy as np

import concourse.bass as bass
import concourse.tile as tile
from concourse import bacc, bass_utils, dve_ops, mybir
from concourse.dve_spec import AluOp, Spec, Src0, Src1, _has_src1, lower, scan
from concourse.dve_uop import DveOpSpec
from concourse.masks import make_identity

# Problem shape (hardcoded per contract; kernel.py must be self-contained).
B, E, A, D, K = 128, 32, 16, 512, 128
N_CORES = 8
B_SH = B // N_CORES          # batches per core = 16
BE = B_SH * E                # rows per core = 512
NBC = BE // 128              # be-chunks of 128 partitions = 4
NDC = D // 128               # din-chunks = 4
HALF_A = 8                   # visible-agents per streamed tile (max)
F32 = mybir.dt.float32


# ---- custom DVE op: out = cumsum(in0 * in1) along the free axis ---------- #

def _ref_cumsum_mul(in0, in1, s0, s1, imm2):
    p = in0.shape[0]
    a = np.asarray(in0, np.float32).reshape(p, -1)
    b = np.ascontiguousarray(np.asarray(in1, np.float32)).reshape(p, -1)
    if b.shape[1] != a.shape[1]:
        b = np.tile(b, (1, a.shape[1] // b.shape[1]))
    init = s0 if isinstance(s0, np.ndarray) else np.float32(s0)
    return init + np.cumsum(a * b, axis=-1, dtype=np.float32)


def _register_cumsum_mul():
    name = "CUMSUM_MUL_ANT"
    if name in dve_ops._SUB_OPCODE_FOR_NAME:
        return next(op for op in dve_ops.OPS if op.name == name)
    from concourse.dve_spec import C0
    spec = Spec(body=scan(AluOp.ADD, Src0 * Src1, init=C0), reference=_ref_cumsum_mul)
    row = dve_ops._CUSTOM_DVE_ROW_BASE + len(dve_ops.OPS)
    assert row < 0x20
    shas = {}
    for ver in ("v3", "v4"):
        d = DveOpSpec(name=name, opcode=row, uops=lower(spec, ver=ver),
                      rd1_en=_has_src1(spec))
        shas[ver] = d.sha(ver)
    op = dve_ops.DveOp(name, spec, subdim=False, uops_sha=shas)
    dve_ops._SUB_OPCODE_FOR_NAME[name] = row
    dve_ops.OPS.append(op)
    dve_ops.CUSTOM_DVE_SPECS[name] = spec
    return op


CUMSUM_MUL = _register_cumsum_mul()


def _bcast_mid(ap_2d, count):
    """[P, N] AP -> [P, count, N] AP with a step-0 middle dim."""
    return bass.AP(
        tensor=ap_2d.tensor,
        offset=ap_2d.offset,
        ap=[ap_2d.ap[0], [0, count], *ap_2d.ap[1:]],
    )


def _emit(tc, nc, ag_d, vis_d, wpsi_d, wphi_d, out_d):
    with ExitStack() as ctx:
        const = ctx.enter_context(tc.tile_pool(name="const", bufs=1))
        agp = ctx.enter_context(tc.tile_pool(name="agp", bufs=1))
        visp = ctx.enter_context(tc.tile_pool(name="visp", bufs=6))
        cump = ctx.enter_context(tc.tile_pool(name="cump", bufs=2))
        small = ctx.enter_context(tc.tile_pool(name="small", bufs=4))
        ps_tr = ctx.enter_context(tc.tile_pool(name="ps_tr", bufs=4, space="PSUM"))
        ps_mm = ctx.enter_context(tc.tile_pool(name="ps_mm", bufs=2, space="PSUM"))

        ident = const.tile([128, 128], F32)
        make_identity(nc, ident)
        # Pay GpSimd's one-time TENSOR_TENSOR ucode load now, while the Pool
        # engine is idle, not at the first beta extraction.
        scratch = small.tile([128, 2], F32, tag="scr", name="scr")
        nc.gpsimd.tensor_sub(scratch[:, 0:1], ident[:, 0:1], ident[:, 1:2])
        nc.gpsimd.tensor_copy(scratch[:, 1:2], ident[:, 0:1])

        # Two HWDGE rings: the SP (sync) ring carries ONLY the 16 MB vis
        # stream, so its ramp is not polluted by small transfers; all small
        # loads (agent chunks + both weights) ride the ACT (scalar) ring in
        # parallel during the ramp.  Ring order IS arrival order per ring.
        # Weights use interleaved din chunking (chunk r = rows d % 4 == r)
        # so each partition line is a contiguous 2 KB DMA.
        ag_tiles = {}
        for c in range(NBC):
            ag_tiles[c] = agp.tile([128, D], F32, tag=f"ag{c}", name=f"ag{c}")
        nc.scalar.dma_start(out=ag_tiles[0], in_=ag_d[0:128, :])
        wpsi_sb = const.tile([128, NDC, K], F32)
        nc.scalar.dma_start(out=wpsi_sb, in_=wpsi_d.rearrange("(p r) k -> p r k", r=NDC))
        wphi_sb = const.tile([128, NDC, K], F32)
        nc.scalar.dma_start(out=wphi_sb, in_=wphi_d.rearrange("(p r) k -> p r k", r=NDC))
        for c in range(1, NBC):
            nc.scalar.dma_start(out=ag_tiles[c], in_=ag_d[c * 128:(c + 1) * 128, :])

        # vis DMA issue pass. First/last chunks use finer tiles to shorten
        # the pipeline ramp and tail.
        chunk_groups = {0: [4, 4, 8], 1: [8, 8], 2: [8, 8], 3: [8, 4, 2, 2]}
        vis_tiles = {}
        for c in range(NBC):
            cs = slice(c * 128, (c + 1) * 128)
            a0 = 0
            for gi, na in enumerate(chunk_groups[c]):
                vis_sb = visp.tile([128, HALF_A, D], F32, tag="vis",
                                   name=f"vis{c}_{gi}")[:, :na, :]
                nc.sync.dma_start(
                    out=vis_sb, in_=vis_d[cs, a0 * D:(a0 + na) * D]
                )
                vis_tiles[(c, gi)] = (vis_sb, a0, na)
                a0 += na

        # Warm the PE clock (HAM) with dummy transposes, bridging the gap
        # until ag0 lands so the governor sees sustained PE load and the
        # t[0] chain runs above the cold 1.2 GHz.
        for wup in range(7):
            warm_ps = ps_tr.tile([128, 128], F32, tag="tr", name=f"warm{wup}")
            nc.tensor.transpose(warm_ps, ident, ident)

        # PE prologue per chunk: agT transposes -> qT -> t.  Chunk 0 is the
        # latency-critical chain gating the first scan: its PSUM
        # evacuations run on the (idle until then) Vector engine; later
        # chunks' run on Scalar so the DVE stays free for scans.
        # w_phiT with natural dout order: wphiT4[k, dl, r] = w_phi[4*dl+r, k],
        # flat free index f = dl*4 + r = dout.
        agT_sb = const.tile([128, NDC, BE], F32)
        qT_sb = const.tile([128, BE], F32)
        wphiT_sb = const.tile([128, 128, NDC], F32)
        t_tiles = []
        for c in range(NBC):
            cs = slice(c * 128, (c + 1) * 128)
            copy_eng = nc.vector.tensor_copy if c == 0 else nc.scalar.copy
            ag_v = ag_tiles[c].rearrange("p (q r) -> p q r", r=NDC)
            for r in range(NDC):
                tr_ps = ps_tr.tile([128, 128], F32, tag="tr", name=f"tra{c}_{r}")
                nc.tensor.transpose(tr_ps, ag_v[:, :, r], ident)
                copy_eng(agT_sb[:, r, cs], tr_ps)
            # qT[:, cs] = sum_r w_psi_chunk_r.T @ agT_chunk_r
            qt_ps = ps_mm.tile([128, 128], F32, tag="qt", name=f"qt{c}")
            for r in range(NDC):
                nc.tensor.matmul(
                    qt_ps,
                    lhsT=wpsi_sb[:, r, :],
                    rhs=agT_sb[:, r, cs],
                    start=(r == 0),
                    stop=(r == NDC - 1),
                )
            copy_eng(qT_sb[:, cs], qt_ps)
            if c == 0:
                # wphiT transposes sit between qT and the t matmul on the PE
                # queue: they are only needed for t, and this keeps the agT
                # chain (gated by the ag0 DMA) at the front of the queue.
                for r in range(NDC):
                    tr_ps = ps_tr.tile([128, 128], F32, tag="tr", name=f"trw{r}")
                    nc.tensor.transpose(tr_ps, wphi_sb[:, r, :], ident)
                    nc.scalar.copy(wphiT_sb[:, :, r], tr_ps)
            # t[be_c, dout] = qT[:, cs].T @ w_phiT
            t_ps = ps_mm.tile([128, D], F32, tag="t", name=f"tps{c}")
            nc.tensor.matmul(
                t_ps, lhsT=qT_sb[:, cs], rhs=wphiT_sb[:, :, :], start=True, stop=True
            )
            t_sb = const.tile([128, D], F32, tag=f"t{c}", name=f"t{c}")
            copy_eng(t_sb, t_ps)
            t_tiles.append(t_sb)

        # Streaming phase.  The DVE runs (almost) only scans: each group's
        # multiply+cumsum writes OUT-OF-PLACE into a per-chunk cum region,
        # seeded (s0) with the previous group's final cumsum so the chunk's
        # 16 boundary values form one running series.  The vis tile's only
        # reader is then the scan itself, so the DMA ring's WAR release is
        # immediate.  GpSimd does one copy+sub per chunk on the cum region
        # (its lag behind scans — the Pool SBUF port is blocked while a
        # 2-src DVE op runs — is harmless there).  Scalar: exp + mul + out.
        # The tiny DVE reduce/reciprocal are emitted one chunk late so they
        # never block the scan queue head.
        beta_t, negm_t, prob_t, ssum_t, osb_t = {}, {}, {}, {}, {}
        for c in range(NBC):
            beta_t[c] = small.tile([128, A], F32, tag="beta", name=f"beta{c}")
            negm_t[c] = small.tile([128, 1], F32, tag="negm", name=f"negm{c}")
            prob_t[c] = small.tile([128, A], F32, tag="prob", name=f"prob{c}")
            ssum_t[c] = small.tile([128, 1], F32, tag="ssum", name=f"ssum{c}")
            osb_t[c] = small.tile([128, A], F32, tag="osb", name=f"osb{c}")

        def softmax_head(c):
            nc.vector.tensor_reduce(
                negm_t[c], beta_t[c], axis=mybir.AxisListType.X,
                op=mybir.AluOpType.max, negate=True,
            )
            nc.scalar.activation(
                prob_t[c], beta_t[c], mybir.ActivationFunctionType.Exp,
                bias=negm_t[c], scale=1.0, accum_out=ssum_t[c],
            )

        def softmax_tail(c):
            rec = small.tile([128, 1], F32, tag="rec", name=f"rec{c}")
            nc.vector.reciprocal(rec, ssum_t[c])
            nc.scalar.mul(osb_t[c], prob_t[c], rec)
            cs = slice(c * 128, (c + 1) * 128)
            nc.scalar.dma_start(out=out_d[cs, :], in_=osb_t[c])

        for c in range(NBC):
            cum_sb = cump.tile([128, A, D], F32, tag="cum", name=f"cum{c}")
            cum_flat = cum_sb.rearrange("p a d -> p (a d)")
            for gi in range(len(chunk_groups[c])):
                vis_sb, a0, na = vis_tiles[(c, gi)]
                seed = 0.0 if a0 == 0 else cum_flat[:, a0 * D - 1:a0 * D]
                nc.vector._custom_dve(
                    CUMSUM_MUL,
                    out=cum_sb[:, a0:a0 + na, :],
                    in0=vis_sb,
                    in1=_bcast_mid(t_tiles[c], na),
                    s0=seed,
                )
                if c >= 1 and gi == 0:
                    softmax_head(c - 1)
                if c >= 1 and gi == 1:
                    softmax_tail(c - 1)
            # running-cumsum boundaries cum[:, a, D-1] -> [128, A]
            bnd = cum_sb[:, :, D - 1:D].rearrange("p a o -> p (a o)")
            nc.gpsimd.tensor_copy(beta_t[c][:, 0:1], bnd[:, 0:1])
            nc.gpsimd.tensor_sub(beta_t[c][:, 1:A], bnd[:, 1:A], bnd[:, 0:A - 1])
        softmax_head(NBC - 1)
        softmax_tail(NBC - 1)


def _build_program():
    nc = bacc.Bacc("TRN2", target_bir_lowering=False, debug=False)
    ag_d = nc.dram_tensor("agent", (BE, D), F32, kind="ExternalInput").ap()
    vis_d = nc.dram_tensor("vis", (BE, A * D), F32, kind="ExternalInput").ap()
    wpsi_d = nc.dram_tensor("w_psi", (D, K), F32, kind="ExternalInput").ap()
    wphi_d = nc.dram_tensor("w_phi", (D, K), F32, kind="ExternalInput").ap()
    out_d = nc.dram_tensor("out", (BE, A), F32, kind="ExternalOutput").ap()
    with tile.TileContext(nc) as tc:
        _emit(tc, nc, ag_d, vis_d, wpsi_d, wphi_d, out_d)
    nc.compile()
    return nc


_PROG = None


def _get_program():
    global _PROG
    if _PROG is None:
        _PROG = _build_program()
    return _PROG


def make_in_maps(agent_observation, visible_observations, w_psi, w_phi):
    agent = np.ascontiguousarray(np.asarray(agent_observation, np.float32)).reshape(B, E, D)
    vis = np.ascontiguousarray(np.asarray(visible_observations, np.float32)).reshape(B, E, A, D)
    wpsi = np.ascontiguousarray(np.asarray(w_psi, np.float32))
    wphi = np.ascontiguousarray(np.asarray(w_phi, np.float32))
    in_maps = []
    for ci in range(N_CORES):
        sl = slice(ci * B_SH, (ci + 1) * B_SH)
        in_maps.append({
            "agent": np.ascontiguousarray(agent[sl].reshape(BE, D)),
            "vis": np.ascontiguousarray(vis[sl].reshape(BE, A * D)),
            "w_psi": wpsi,
            "w_phi": wphi,
        })
    return in_maps


def run_sharded(in_maps, trace=False, **kwargs):
    nc = _get_program()
    return bass_utils.run_bass_kernel_spmd(
        nc, in_maps, core_ids=list(range(N_CORES)), trace=trace, **kwargs
    )


def kernel(agent_observation, visible_observations, w_psi, w_phi):
    in_maps = make_in_maps(agent_observation, visible_observations, w_psi, w_phi)
    res = run_sharded(in_maps)
    return np.concatenate(
        [r["out"].reshape(B_SH, E, A) for r in res.results], axis=0
    )


# revision 8
# speedup vs baseline: 1.3700x; 1.0096x over previous
"""Trainium2 Bass kernel for EntityAttention.

    beta[b,e,a] = (agent[b,e] @ w_psi) . (vis[b,e,a] @ w_phi)
    out         = softmax_a(beta)

Refactored so the huge `visible_observations` tensor is read exactly once,
in its natural layout, by a fused multiply+cumulative-sum on the Vector
engine (a custom DVE op: out = cumsum(in0 * in1)); per-a segment sums are
recovered by differencing the cumsum at segment boundaries:

    qT[k, be]   = sum_din w_psi[din, k] * agent[be, din]      (PE)
    t[be, dout] = sum_k   qT[k, be]     * w_phiT[k, dout]     (PE)
    cum         = cumsum_f(vis[be, (a,d)] * t[be, d bcast])   (DVE, 1 op / 8 a)
    beta[be, a] = cum[.., a*D+D-1] - cum[.., a*D-1]           (GpSimd, tiny)
    out[be, a]  = softmax_a(beta)                             (DVE max + ACT
                                                               exp + GpSimd
                                                               normalize)

Sharding: data-parallel over the batch axis across 8 NeuronCores
(16 batches / core); w_psi / w_phi replicated.

Engine budget in steady state: the DVE queue is (almost) pure scans so it
keeps pace with the ~430 GB/s HBM stream; beta extraction and softmax
normalization run on the otherwise-idle GpSimd engine; PSUM evacuations on
the chunk-0 critical path run on the (idle until first scan) Vector engine,
later chunks' on Scalar.
"""

from contextlib import ExitStack

import numpy as np

import concourse.bass as bass
import concourse.tile as tile
from concourse import bacc, bass_utils, dve_ops, mybir
from concourse.dve_spec import AluOp, Spec, Src0, Src1, _has_src1, lower, scan
from concourse.dve_uop import DveOpSpec
from concourse.masks import make_identity

# Problem shape (hardcoded per contract; kernel.py must be self-contained).
B, E, A, D, K = 128, 32, 16, 512, 128
N_CORES = 8
B_SH = B // N_CORES          # batches per core = 16
BE = B_SH * E                # rows per core = 512
NBC = BE // 128              # be-chunks of 128 partitions = 4
NDC = D // 128               # din-chunks = 4
HALF_A = 8                   # visible-agents per streamed tile (max)
F32 = mybir.dt.float32


# ---- custom DVE op: out = cumsum(in0 * in1) along the free axis ---------- #

def _ref_cumsum_mul(in0, in1, s0, s1, imm2):
    p = in0.shape[0]
    a = np.asarray(in0, np.float32).reshape(p, -1)
    b = np.ascontiguousarray(np.asarray(in1, np.float32)).reshape(p, -1)
    if b.shape[1] != a.shape[1]:
        b = np.tile(b, (1, a.shape[1] // b.shape[1]))
    init = s0 if isinstance(s0, np.ndarray) else np.float32(s0)
    return init + np.cumsum(a * b, axis=-1, dtype=np.float32)


def _register_cumsum_mul():
    name = "CUMSUM_MUL_ANT"
    if name in dve_ops._SUB_OPCODE_FOR_NAME:
        return next(op for op in dve_ops.OPS if op.name == name)
    from concourse.dve_spec import C0
    spec = Spec(body=scan(AluOp.ADD, Src0 * Src1, init=C0), reference=_ref_cumsum_mul)
    row = dve_ops._CUSTOM_DVE_ROW_BASE + len(dve_ops.OPS)
    assert row < 0x20
    shas = {}
    for ver in ("v3", "v4"):
        d = DveOpSpec(name=name, opcode=row, uops=lower(spec, ver=ver),
                      rd1_en=_has_src1(spec))
        shas[ver] = d.sha(ver)
    op = dve_ops.DveOp(name, spec, subdim=False, uops_sha=shas)
    dve_ops._SUB_OPCODE_FOR_NAME[name] = row
    dve_ops.OPS.append(op)
    dve_ops.CUSTOM_DVE_SPECS[name] = spec
    return op


CUMSUM_MUL = _register_cumsum_mul()


def _bcast_mid(ap_2d, count):
    """[P, N] AP -> [P, count, N] AP with a step-0 middle dim."""
    return bass.AP(
        tensor=ap_2d.tensor,
        offset=ap_2d.offset,
        ap=[ap_2d.ap[0], [0, count], *ap_2d.ap[1:]],
    )


def _emit(tc, nc, ag_d, vis_d, wpsi_d, wphi_d, out_d):
    with ExitStack() as ctx:
        const = ctx.enter_context(tc.tile_pool(name="const", bufs=1))
        agp = ctx.enter_context(tc.tile_pool(name="agp", bufs=1))
        visp = ctx.enter_context(tc.tile_pool(name="visp", bufs=6))
        cump = ctx.enter_context(tc.tile_pool(name="cump", bufs=2))
        small = ctx.enter_context(tc.tile_pool(name="small", bufs=4))
        ps_tr = ctx.enter_context(tc.tile_pool(name="ps_tr", bufs=4, space="PSUM"))
        ps_mm = ctx.enter_context(tc.tile_pool(name="ps_mm", bufs=2, space="PSUM"))

        ident = const.tile([128, 128], F32)
        make_identity(nc, ident)
        # Pay GpSimd's one-time TENSOR_TENSOR ucode load now, while the Pool
        # engine is idle, not at the first beta extraction.
        scratch = small.tile([128, 2], F32, tag="scr", name="scr")
        nc.gpsimd.tensor_sub(scratch[:, 0:1], ident[:, 0:1], ident[:, 1:2])
        nc.gpsimd.tensor_copy(scratch[:, 1:2], ident[:, 0:1])

        # DMA issue order on the SP (sync) HWDGE ring IS the stream order.
        # The three t[0]-gating loads go first (they must beat the 16 MB vis
        # flood — a separate ring would starve behind it); agent chunk c is
        # issued just ahead of the vis tiles whose scans need t[c].
        # Weights use interleaved din chunking (chunk r = rows d % 4 == r)
        # so each partition line is a contiguous 2 KB DMA.
        ag_tiles = {}
        for c in range(NBC):
            ag_tiles[c] = agp.tile([128, D], F32, tag=f"ag{c}", name=f"ag{c}")
        wphi_sb = const.tile([128, NDC, K], F32)
        nc.sync.dma_start(out=wphi_sb, in_=wphi_d.rearrange("(p r) k -> p r k", r=NDC))
        nc.sync.dma_start(out=ag_tiles[0], in_=ag_d[0:128, :])
        wpsi_sb = const.tile([128, NDC, K], F32)
        nc.sync.dma_start(out=wpsi_sb, in_=wpsi_d.rearrange("(p r) k -> p r k", r=NDC))

        # vis DMA issue pass. First/last chunks use finer tiles to shorten
        # the pipeline ramp and tail.
        chunk_groups = {0: [4, 4, 8], 1: [8, 8], 2: [8, 8], 3: [8, 4, 2, 2]}
        vis_tiles = {}
        for c in range(NBC):
            cs = slice(c * 128, (c + 1) * 128)
            if c > 0:
                nc.sync.dma_start(
                    out=ag_tiles[c], in_=ag_d[c * 128:(c + 1) * 128, :]
                )
            a0 = 0
            for gi, na in enumerate(chunk_groups[c]):
                vis_sb = visp.tile([128, HALF_A, D], F32, tag="vis",
                                   name=f"vis{c}_{gi}")[:, :na, :]
                nc.sync.dma_start(
                    out=vis_sb, in_=vis_d[cs, a0 * D:(a0 + na) * D]
                )
                vis_tiles[(c, gi)] = (vis_sb, a0, na)
                a0 += na

        # Warm the PE clock (HAM) with dummy transposes, bridging the gap
        # until ag0 lands so the governor sees sustained PE load and the
        # t[0] chain runs above the cold 1.2 GHz.
        for wup in range(7):
            warm_ps = ps_tr.tile([128, 128], F32, tag="tr", name=f"warm{wup}")
            nc.tensor.transpose(warm_ps, ident, ident)

        # PE prologue per chunk: agT transposes -> qT -> t.  Chunk 0 is the
        # latency-critical chain gating the first scan: its PSUM
        # evacuations run on the (idle until then) Vector engine; later
        # chunks' run on Scalar so the DVE stays free for scans.
        # w_phiT with natural dout order: wphiT4[k, dl, r] = w_phi[4*dl+r, k],
        # flat free index f = dl*4 + r = dout.
        agT_sb = const.tile([128, NDC, BE], F32)
        qT_sb = const.tile([128, BE], F32)
        wphiT_sb = const.tile([128, 128, NDC], F32)
        t_tiles = []
        for c in range(NBC):
            cs = slice(c * 128, (c + 1) * 128)
            copy_eng = nc.vector.tensor_copy if c == 0 else nc.scalar.copy
            ag_v = ag_tiles[c].rearrange("p (q r) -> p q r", r=NDC)
            for r in range(NDC):
                tr_ps = ps_tr.tile([128, 128], F32, tag="tr", name=f"tra{c}_{r}")
                nc.tensor.transpose(tr_ps, ag_v[:, :, r], ident)
                copy_eng(agT_sb[:, r, cs], tr_ps)
            # qT[:, cs] = sum_r w_psi_chunk_r.T @ agT_chunk_r
            qt_ps = ps_mm.tile([128, 128], F32, tag="qt", name=f"qt{c}")
            for r in range(NDC):
                nc.tensor.matmul(
                    qt_ps,
                    lhsT=wpsi_sb[:, r, :],
                    rhs=agT_sb[:, r, cs],
                    start=(r == 0),
                    stop=(r == NDC - 1),
                )
            copy_eng(qT_sb[:, cs], qt_ps)
            if c == 0:
                # wphiT transposes sit between qT and the t matmul on the PE
                # queue: they are only needed for t, and this keeps the agT
                # chain (gated by the ag0 DMA) at the front of the queue.
                for r in range(NDC):
                    tr_ps = ps_tr.tile([128, 128], F32, tag="tr", name=f"trw{r}")
                    nc.tensor.transpose(tr_ps, wphi_sb[:, r, :], ident)
                    nc.scalar.copy(wphiT_sb[:, :, r], tr_ps)
            # t[be_c, dout] = qT[:, cs].T @ w_phiT
            t_ps = ps_mm.tile([128, D], F32, tag="t", name=f"tps{c}")
            nc.tensor.matmul(
                t_ps, lhsT=qT_sb[:, cs], rhs=wphiT_sb[:, :, :], start=True, stop=True
            )
            t_sb = const.tile([128, D], F32, tag=f"t{c}", name=f"t{c}")
            copy_eng(t_sb, t_ps)
            t_tiles.append(t_sb)

        # Streaming phase.  The DVE runs (almost) only scans: each group's
        # multiply+cumsum writes OUT-OF-PLACE into a per-chunk cum region,
        # seeded (s0) with the previous group's final cumsum so the chunk's
        # 16 boundary values form one running series.  The vis tile's only
        # reader is then the scan itself, so the DMA ring's WAR release is
        # immediate.  GpSimd does one copy+sub per chunk on the cum region
        # (its lag behind scans — the Pool SBUF port is blocked while a
        # 2-src DVE op runs — is harmless there).  Scalar: exp + mul + out.
        # The tiny DVE reduce/reciprocal are emitted one chunk late so they
        # never block the scan queue head.
        beta_t, negm_t, prob_t, ssum_t, osb_t = {}, {}, {}, {}, {}
        for c in range(NBC):
            beta_t[c] = small.tile([128, A], F32, tag="beta", name=f"beta{c}")
            negm_t[c] = small.tile([128, 1], F32, tag="negm", name=f"negm{c}")
            prob_t[c] = small.tile([128, A], F32, tag="prob", name=f"prob{c}")
            ssum_t[c] = small.tile([128, 1], F32, tag="ssum", name=f"ssum{c}")
            osb_t[c] = small.tile([128, A], F32, tag="osb", name=f"osb{c}")

        def softmax_head(c):
            nc.vector.tensor_reduce(
                negm_t[c], beta_t[c], axis=mybir.AxisListType.X,
                op=mybir.AluOpType.max, negate=True,
            )
            nc.scalar.activation(
                prob_t[c], beta_t[c], mybir.ActivationFunctionType.Exp,
                bias=negm_t[c], scale=1.0, accum_out=ssum_t[c],
            )

        def softmax_tail(c):
            rec = small.tile([128, 1], F32, tag="rec", name=f"rec{c}")
            nc.vector.reciprocal(rec, ssum_t[c])
            nc.scalar.mul(osb_t[c], prob_t[c], rec)
            cs = slice(c * 128, (c + 1) * 128)
            nc.scalar.dma_start(out=out_d[cs, :], in_=osb_t[c])

        for c in range(NBC):
            cum_sb = cump.tile([128, A, D], F32, tag="cum", name=f"cum{c}")
            cum_flat = cum_sb.rearrange("p a d -> p (a d)")
            for gi in range(len(chunk_groups[c])):
                vis_sb, a0, na = vis_tiles[(c, gi)]
                seed = 0.0 if a0 == 0 else cum_flat[:, a0 * D - 1:a0 * D]
                nc.vector._custom_dve(
                    CUMSUM_MUL,
                    out=cum_sb[:, a0:a0 + na, :],
                    in0=vis_sb,
                    in1=_bcast_mid(t_tiles[c], na),
                    s0=seed,
                )
            # running-cumsum boundaries cum[:, a, D-1] -> [128, A]
            bnd = cum_sb[:, :, D - 1:D].rearrange("p a o -> p (a o)")
            nc.gpsimd.tensor_copy(beta_t[c][:, 0:1], bnd[:, 0:1])
            nc.gpsimd.tensor_sub(beta_t[c][:, 1:A], bnd[:, 1:A], bnd[:, 0:A - 1])
            softmax_head(c)
            softmax_tail(c)


def _build_program():
    nc = bacc.Bacc("TRN2", target_bir_lowering=False, debug=False)
    ag_d = nc.dram_tensor("agent", (BE, D), F32, kind="ExternalInput").ap()
    vis_d = nc.dram_tensor("vis", (BE, A * D), F32, kind="ExternalInput").ap()
    wpsi_d = nc.dram_tensor("w_psi", (D, K), F32, kind="ExternalInput").ap()
    wphi_d = nc.dram_tensor("w_phi", (D, K), F32, kind="ExternalInput").ap()
    out_d = nc.dram_tensor("out", (BE, A), F32, kind="ExternalOutput").ap()
    with tile.TileContext(nc) as tc:
        _emit(tc, nc, ag_d, vis_d, wpsi_d, wphi_d, out_d)
    nc.compile()
    return nc


_PROG = None


def _get_program():
    global _PROG
    if _PROG is None:
        _PROG = _build_program()
    return _PROG


def make_in_maps(agent_observation, visible_observations, w_psi, w_phi):
    agent = np.ascontiguousarray(np.asarray(agent_observation, np.float32)).reshape(B, E, D)
    vis = np.ascontiguousarray(np.asarray(visible_observations, np.float32)).reshape(B, E, A, D)
    wpsi = np.ascontiguousarray(np.asarray(w_psi, np.float32))
    wphi = np.ascontiguousarray(np.asarray(w_phi, np.float32))
    in_maps = []
    for ci in range(N_CORES):
        sl = slice(ci * B_SH, (ci + 1) * B_SH)
        in_maps.append({
            "agent": np.ascontiguousarray(agent[sl].reshape(BE, D)),
            "vis": np.ascontiguousarray(vis[sl].reshape(BE, A * D)),
            "w_psi": wpsi,
            "w_phi": wphi,
        })
    return in_maps


def run_sharded(in_maps, trace=False, **kwargs):
    nc = _get_program()
    return bass_utils.run_bass_kernel_spmd(
        nc, in_maps, core_ids=list(range(N_CORES)), trace=trace, **kwargs
    )


def kernel(agent_observation, visible_observations, w_psi, w_phi):
    in_maps = make_in_maps(agent_observation, visible_observations, w_psi, w_phi)
    res = run_sharded(in_maps)
    return np.concatenate(
        [r["out"].reshape(B_SH, E, A) for r in res.results], axis=0
    )
